# revision 7
# baseline (speedup 1.0000x reference)
"""BahdanauAttentionAudio TRN2 kernel — 8-core batch-sharded.

Pipeline per core (16 batch rows):
  ph0: convo = prev_att @ Wc_center.T (f32 PE), qproj+biases (f32 PE)
  ph1: sum1 = values_t.T @ W1T + bias + locproj x convo (f32 PE, rank-1 fold),
       tanh (ACT, fused bias), score = Vw . tanh (f32 PE) -> score_sb [16,2048]
  ph2: top-K threshold via 24-iter DVE bisection (3 ops/iter), masked score,
       sigmoid (ACT), partial sigma_b sig (PE ones-reduce), AllReduce over 8
       cores, att = sig / sigsum (PE bcast + DVE)
  ph3: att transposed to [128t,(c,b)] bf16 via PE transposes
  ph4: context = sum_t att*values (bf16 PE, values_bf16 host-prepped)
Host preps transposed/padded tensors; outputs gathered on host.
"""
import numpy as np
import ml_dtypes
import concourse.bass as bass
import concourse.bacc as bacc
import concourse.tile as tile
from concourse import mybir
from concourse.bass_utils import run_bass_kernel_spmd

f32 = mybir.dt.float32
bf16 = mybir.dt.bfloat16

B, T, H, U = 128, 1998, 256, 256
K = 1332
TP = 2048            # padded T
NCORES = 8
RB = B // NCORES     # 16 rows per core
NIT = 24             # bisection iterations
DELTA = 1e-6         # final threshold slack (validated vs min rank-K gap 2.2e-6)
NEG = -1.0e9


def build(v_b: float, reps: int = 1):
    nc = bacc.Bacc(None, target_bir_lowering=False)

    vt_d = nc.dram_tensor("vt", [RB, H, TP], f32, kind="ExternalInput")
    vbf_d = nc.dram_tensor("vbf", [RB, TP, H], bf16, kind="ExternalInput")
    wct_d = nc.dram_tensor("wct", [TP, TP], f32, kind="ExternalInput")
    pat_d = nc.dram_tensor("pat", [128, 16 * RB], f32, kind="ExternalInput")
    qt_d = nc.dram_tensor("qt", [128, 2 * RB], f32, kind="ExternalInput")
    w1t_d = nc.dram_tensor("w1t", [128, 2 * U], f32, kind="ExternalInput")
    w2t_d = nc.dram_tensor("w2t", [128, 2 * U], f32, kind="ExternalInput")
    biasrow_d = nc.dram_tensor("biasrow", [1, U], f32, kind="ExternalInput")
    locp_d = nc.dram_tensor("locp", [1, U], f32, kind="ExternalInput")
    vw_d = nc.dram_tensor("vw", [128, 2], f32, kind="ExternalInput")
    id16_d = nc.dram_tensor("id16", [16, 16], f32, kind="ExternalInput")

    score_o = nc.dram_tensor("score_o", [RB, T], f32, kind="ExternalOutput")
    att_o = nc.dram_tensor("att_o", [RB, T], f32, kind="ExternalOutput")
    ctx_o = nc.dram_tensor("ctx_o", [RB, H], f32, kind="ExternalOutput")

    cc_in = nc.dram_tensor("cc_in", [1, TP], f32)
    cc_out = nc.dram_tensor("cc_out", [1, TP], f32, addr_space="Shared")

    with tile.TileContext(nc) as tc:
        with (
            tc.tile_pool(name="const", bufs=1) as cpool,
            tc.tile_pool(name="persist", bufs=1) as pers,
            tc.tile_pool(name="stream", bufs=4) as strm,
            tc.tile_pool(name="vbfs", bufs=4) as vbfs,
            tc.tile_pool(name="small", bufs=1) as sml,
            tc.tile_pool(name="cvrow", bufs=3) as cvrow,
            tc.tile_pool(name="ps_main", bufs=3, space="PSUM") as psm,
            tc.tile_pool(name="ps_aux", bufs=2, space="PSUM") as psa,
        ):
            # ---- constants to SBUF ----
            w1t = cpool.tile([128, 2 * U], f32)
            nc.sync.dma_start(w1t[:], w1t_d[:])
            w2t = cpool.tile([128, 2 * U], f32)
            nc.sync.dma_start(w2t[:], w2t_d[:])
            qt = cpool.tile([128, 2 * RB], f32)
            nc.sync.dma_start(qt[:], qt_d[:])
            pat = cpool.tile([128, 16 * RB], f32)
            nc.sync.dma_start(pat[:], pat_d[:])
            biasrow = cpool.tile([1, U], f32)
            nc.sync.dma_start(biasrow[:], biasrow_d[:])
            locp = cpool.tile([1, U], f32)
            nc.sync.dma_start(locp[:], locp_d[:])
            vw = cpool.tile([128, 2], f32)
            nc.sync.dma_start(vw[:], vw_d[:])
            id16 = cpool.tile([16, 16], f32)
            nc.sync.dma_start(id16[:], id16_d[:])
            ones16 = cpool.tile([16, 1], f32)
            nc.vector.memset(ones16[:], 1.0)
            ones1x16 = cpool.tile([1, RB], f32)
            nc.vector.memset(ones1x16[:], 1.0)

            for _rep in range(reps):
                # ================= ph0: convo + qproj/bias =================
                convo_sb = pers.tile([RB, TP], f32, tag="convo")
                for oc in range(4):
                    pcv = psa.tile([RB, 512], f32, tag="aux")
                    for ic in range(16):
                        wt = strm.tile([128, 512], f32, tag="wct")
                        nc.sync.dma_start(
                            wt[:], wct_d[ic * 128:(ic + 1) * 128,
                                          oc * 512:(oc + 1) * 512])
                        nc.tensor.matmul(pcv[:], pat[:, ic * RB:(ic + 1) * RB],
                                         wt[:], start=(ic == 0), stop=(ic == 15))
                    nc.vector.tensor_copy(convo_sb[:, oc * 512:(oc + 1) * 512],
                                          pcv[:])

                # qproj+bias: [RB, U] = qt.T @ w2t + ones x biasrow
                pq = psa.tile([RB, U], f32, tag="aux")
                for hc in range(2):
                    nc.tensor.matmul(pq[:], qt[:, hc * RB:(hc + 1) * RB],
                                     w2t[:, hc * U:(hc + 1) * U],
                                     start=(hc == 0), stop=False)
                nc.tensor.matmul(pq[:], ones1x16[:], biasrow[:],
                                 start=False, stop=True)
                bias_bu = sml.tile([RB, U], f32, tag="bias_bu")
                nc.vector.tensor_copy(bias_bu[:], pq[:])
                # transpose to [128u, 16b] per u-chunk for ACT bias
                bias_ub = sml.tile([128, 2 * RB], f32, tag="bias_ub")
                for uc in range(2):
                    pt = psa.tile([128, RB], f32, tag="aux")
                    nc.tensor.transpose(pt[:], bias_bu[:, uc * 128:(uc + 1) * 128],
                                        id16[:])
                    nc.vector.tensor_copy(bias_ub[:, uc * RB:(uc + 1) * RB], pt[:])

                # ================= ph1: sum1 + tanh + score =================
                score_sb = pers.tile([RB, TP], f32, tag="score")
                for b in range(RB):
                    crow = cvrow.tile([1, TP], f32, tag="crow")
                    nc.sync.dma_start(crow[:], convo_sb[b:b + 1, :])
                    for t in range(4):
                        psc = psa.tile([1, 512], f32, tag="psc")
                        for uc in range(2):
                            ps1 = psm.tile([128, 512], f32, tag="ps1")
                            for hc in range(2):
                                xt = strm.tile([128, 512], f32, tag="vt")
                                nc.sync.dma_start(
                                    xt[:], vt_d[b, hc * 128:(hc + 1) * 128,
                                                t * 512:(t + 1) * 512])
                                nc.tensor.matmul(
                                    ps1[:],
                                    w1t[:, hc * U + uc * 128:
                                        hc * U + uc * 128 + 128],
                                    xt[:], start=(hc == 0), stop=False)
                            nc.tensor.matmul(
                                ps1[:], locp[:, uc * 128:(uc + 1) * 128],
                                crow[:, t * 512:(t + 1) * 512],
                                start=False, stop=True)
                            th = strm.tile([128, 512], f32, tag="tanh")
                            nc.scalar.activation(
                                th[:], ps1[:], mybir.ActivationFunctionType.Tanh,
                                bias=bias_ub[:, uc * RB + b:uc * RB + b + 1])
                            nc.tensor.matmul(psc[:], vw[:, uc:uc + 1], th[:],
                                             start=(uc == 0), stop=(uc == 1))
                        scst = strm.tile([1, 512], f32, tag="scst")
                        nc.vector.tensor_copy(scst[:], psc[:])
                        nc.sync.dma_start(
                            score_sb[b:b + 1, t * 512:(t + 1) * 512], scst[:])
                # + V_b on real cols, pad cols to NEG
                nc.vector.tensor_scalar_add(score_sb[:, 0:T], score_sb[:, 0:T],
                                            float(v_b))
                nc.vector.memset(score_sb[:, T:TP], NEG)

                # ================= ph2: topk + sigmoid + allreduce ==========
                mid = sml.tile([RB, 1], f32, tag="mid")
                nc.vector.memset(mid[:], 0.0)
                cnt = sml.tile([RB, 1], f32, tag="cnt")
                pm = sml.tile([RB, 1], f32, tag="pm")
                scr = sml.tile([RB, TP], f32, tag="scr")
                for i in range(NIT):
                    nc.vector.tensor_scalar(scr[:], score_sb[:], mid[:], 1.0,
                                            mybir.AluOpType.is_ge,
                                            mybir.AluOpType.mult)
                    nc.vector.reduce_sum(cnt[:], scr[:],
                                         axis=mybir.AxisListType.X)
                    nc.vector.tensor_scalar(pm[:], cnt[:], float(K), 0.5,
                                            mybir.AluOpType.is_ge,
                                            mybir.AluOpType.subtract)
                    upd = float(4.0 * 0.5 ** (i + 1))
                    nc.vector.scalar_tensor_tensor(
                        mid[:], pm[:], 2.0 * upd, mid[:],
                        mybir.AluOpType.mult, mybir.AluOpType.add)
                nc.vector.tensor_scalar_add(
                    mid[:], mid[:], float(-(4.0 * 0.5 ** NIT) - DELTA))
                # masked score = (score >= theta) * score
                nc.vector.scalar_tensor_tensor(
                    score_sb[:], score_sb[:], mid[:], score_sb[:],
                    mybir.AluOpType.is_ge, mybir.AluOpType.mult)
                nc.sync.dma_start(score_o[:], score_sb[:, 0:T])

                sig = pers.tile([RB, TP], f32, tag="sig")
                nc.scalar.activation(sig[:], score_sb[:],
                                     mybir.ActivationFunctionType.Sigmoid)
                # partial sigsum over batch rows (PE ones-reduce)
                sspart = sml.tile([1, TP], f32, tag="sspart")
                for t in range(4):
                    pss = psa.tile([1, 512], f32, tag="psc")
                    nc.tensor.matmul(pss[:], ones16[:],
                                     sig[:, t * 512:(t + 1) * 512],
                                     start=True, stop=True)
                    nc.vector.tensor_copy(sspart[:, t * 512:(t + 1) * 512],
                                          pss[:])
                nc.sync.dma_start(cc_in[:], sspart[:])
                nc.gpsimd.collective_compute(
                    "AllReduce", mybir.AluOpType.add,
                    ins=[cc_in[:]], outs=[cc_out[:]],
                    replica_groups=[list(range(NCORES))])
                ssfull = sml.tile([1, TP], f32, tag="ssfull")
                nc.sync.dma_start(ssfull[:], cc_out[:])
                rec = sml.tile([1, TP], f32, tag="rec")
                nc.vector.reciprocal(rec[:], ssfull[:])
                # broadcast recip to 16 partitions via PE rank-1, per chunk
                att = pers.tile([RB, TP], f32, tag="att")
                for t in range(4):
                    prb = psa.tile([RB, 512], f32, tag="aux")
                    nc.tensor.matmul(prb[:], ones1x16[:],
                                     rec[:, t * 512:(t + 1) * 512],
                                     start=True, stop=True)
                    nc.vector.tensor_tensor(att[:, t * 512:(t + 1) * 512],
                                            sig[:, t * 512:(t + 1) * 512],
                                            prb[:], mybir.AluOpType.mult)
                nc.sync.dma_start(att_o[:], att[:, 0:T])

                # ================= ph3: att -> [128t, (c,b)] bf16 ===========
                att_t = pers.tile([128, 16 * RB], bf16, tag="att_t")
                for c in range(16):
                    pt2 = psa.tile([128, RB], f32, tag="aux")
                    nc.tensor.transpose(pt2[:], att[:, c * 128:(c + 1) * 128],
                                        id16[:])
                    nc.vector.tensor_copy(att_t[:, c * RB:(c + 1) * RB], pt2[:])

                # ================= ph4: context =============================
                ctx_sb = sml.tile([RB, H], f32, tag="ctx")
                for b in range(RB):
                    pctx = psa.tile([1, H], f32, tag="aux")
                    for c in range(16):
                        vb = vbfs.tile([128, H], bf16, tag="vbf")
                        nc.sync.dma_start(
                            vb[:], vbf_d[b, c * 128:(c + 1) * 128, :])
                        nc.tensor.matmul(pctx[:],
                                         att_t[:, c * RB + b:c * RB + b + 1],
                                         vb[:], start=(c == 0), stop=(c == 15))
                    cxst = strm.tile([1, H], f32, tag="cxst")
                    nc.vector.tensor_copy(cxst[:], pctx[:])
                    nc.sync.dma_start(ctx_sb[b:b + 1, :], cxst[:])
                nc.sync.dma_start(ctx_o[:], ctx_sb[:])

    nc.compile()
    return nc


def prep_inputs(query, values, prev_att, W1_w, W1_b, W2_w, W2_b, V_w, V_b,
                conv_w, locproj_w):
    """Host-side sharding + layout prep. Returns (in_maps, v_b_scalar)."""
    in_maps = []
    wct = np.zeros((TP, TP), np.float32)
    wct[:T, :T] = conv_w[:, :, 3].T  # [i, o]
    w1t = np.ascontiguousarray(W1_w.T)  # [h, u]
    w1t_p = np.concatenate([w1t[0:128], w1t[128:256]], axis=1)  # [128, 2U] h-chunks
    w2t = np.ascontiguousarray(W2_w.T)
    w2t_p = np.concatenate([w2t[0:128], w2t[128:256]], axis=1)
    biasrow = (W1_b + W2_b).reshape(1, U).astype(np.float32)
    locp = locproj_w[:, 0].reshape(1, U).astype(np.float32)
    vw_p = np.stack([V_w[0, 0:128], V_w[0, 128:256]], axis=1).astype(np.float32)
    id16 = np.eye(16, dtype=np.float32)

    for cid in range(NCORES):
        rows = slice(cid * RB, (cid + 1) * RB)
        v = values[rows]                                  # [RB, T, H]
        vt = np.zeros((RB, H, TP), np.float32)
        vt[:, :, :T] = np.transpose(v, (0, 2, 1))
        vbf = np.zeros((RB, TP, H), ml_dtypes.bfloat16)
        vbf[:, :T, :] = v.astype(ml_dtypes.bfloat16)
        pa = prev_att[rows, :, 0]                         # [RB, T]
        pat = np.zeros((TP, RB), np.float32)
        pat[:T] = pa.T
        pat_p = np.concatenate([pat[ic * 128:(ic + 1) * 128]
                                for ic in range(16)], axis=1)  # [128, 16*RB]
        q = query[rows]                                   # [RB, H]
        qt = np.ascontiguousarray(q.T)                    # [H, RB]
        qt_p = np.concatenate([qt[0:128], qt[128:256]], axis=1)
        in_maps.append({
            "vt": vt, "vbf": vbf, "wct": wct, "pat": pat_p, "qt": qt_p,
            "w1t": w1t_p, "w2t": w2t_p, "biasrow": biasrow, "locp": locp,
            "vw": vw_p, "id16": id16,
        })
    return in_maps, float(V_b[0])


def kernel(query, values, prev_att, W1_w, W1_b, W2_w, W2_b, V_w, V_b,
           conv_w, locproj_w):
    args = (query, values, prev_att, W1_w, W1_b, W2_w, W2_b, V_w, V_b,
            conv_w, locproj_w)
    args = tuple(np.asarray(a) for a in args)
    in_maps, v_b = prep_inputs(*args)
    nc = build(v_b, reps=1)
    res = run_bass_kernel_spmd(nc, in_maps, core_ids=list(range(NCORES))).results
    ctx = np.concatenate([r["ctx_o"] for r in res], axis=0)
    att = np.concatenate([r["att_o"] for r in res], axis=0)[..., None]
    score = np.concatenate([r["score_o"] for r in res], axis=0)[..., None]
    return ctx.astype(np.float32), att.astype(np.float32), score.astype(np.float32)
